# revision 30
# baseline (speedup 1.0000x reference)
"""MoE layer (8 experts, top-2 sigmoid routing, SwiGLU experts + shared expert)
on 8 TRN2 NeuronCores.

Strategy (expert-parallel, host-side token dispatch):
  - Router (sigmoid(x @ gate_w.T), top-2, weight normalization) is tiny
    (~50 MFLOP) and runs on the host; it determines the all-to-all dispatch.
  - Core c owns expert c: it gets the tokens routed to expert c (gathered and
    zero-padded to a common capacity M_pad) plus expert c's Wi/Wo.
  - The shared expert is data-parallel: core c also processes tokens
    [c*512, (c+1)*512) with the (replicated) shared weights.
  - Device kernel computes the SwiGLU MLP passes with fp32 PSUM
    accumulation, feature-major layout (features on partitions, tokens on
    the free dim) so no on-device transposes are needed. The Wi pass and
    the shared expert run in bf16; the expert Wo pass runs in fp8 e4m3
    with DoubleRow perf mode (2 contraction rows per PE cell, ~1.4x PE
    throughput; rel-err ~1.7e-2 vs the 2e-2 gate, validated by numpy sim).
  - Host combine: out[t] = shared_out[t] + sum_e cw[e,t] * expert_out[e][t]
    (the combine weights, divided by the fp8 weight scale 16, are applied
    on the host during the scatter-add).

Schedule (v3):
  - HBM bandwidth (~320 GB/s) is shared by all DMA queues, so the input
    set streams on ONE priority-ordered sync-HWDGE queue (swi in exact PE
    consumption order, then swo, xe, wi, wo); only the 6 small xs tiles
    ride the scalar queue in parallel so the very first matmul chain has
    its rhs by ~6us (v2's three-way parallel streams split the bandwidth
    and starved the latency-critical shared-expert stream).
  - 6 warm-up matmuls (dep: a gpsimd memset available at ~4.7us) keep the
    PE busy from ~5us so the HAM clock gate un-throttles (4/8 -> 8/8)
    around the time real data lands.
  - Scalar runs ONLY Silu (one ACT table load); PSUM->SBUF output copies
    run on the otherwise-idle GpSimd engine with an f32->bf16 cast.
  - Outputs are bf16 (halves output DMA bytes); each chunk's 6 h-tiles
    leave as two merged 3-h-tile DMAs on gpsimd SWDGE, except the last
    chunk's which ride the (idle, lower-latency) scalar HWDGE queue.
  - Expert chunks are sized so the LAST chunk is ~256 tokens: the tail
    after the final matmul is one small copy + one small DMA + receipt.
  - Emit order Wi0 Wo0 Wi1 Wi2 Wo1 Wi3 Wo2 Wo3: Wo0 uses the
    long-resident shared weights while the expert stream is still in
    flight; afterwards Wi(c+1) precedes Wo(c) so the PE always has
    independent matmul work while ACT/DVE finish a chunk's SwiGLU.
"""

from contextlib import ExitStack

import ml_dtypes
import numpy as np

import concourse.tile as tile
from concourse import bacc, mybir
from concourse.bass_utils import run_bass_kernel_spmd

E, TOPK, H, I = 8, 2, 768, 1152
I2 = 2 * I
T = 4096
N_CORES = 8
TS = T // N_CORES  # shared-expert tokens per core
P = 128
KH = H // P    # 6 contraction tiles over H
KI = I // P    # 9 contraction tiles over I
BF16 = mybir.dt.bfloat16
F32 = mybir.dt.float32
FP8 = mybir.dt.float8e4  # e4m3
MAXN = 512     # max tokens per matmul chunk (one fp32 PSUM bank)
TAILN = 256    # target size of the LAST expert chunk (short drain)

_BUILD_CACHE: dict = {}
LAST_RESULTS = None  # BassKernelResults of the most recent device run
USE_SILU = True  # native ACT Silu on HW; set False for CoreSim (not implemented there)


def _ensure_axon_ntff_hook():
    """This image's `antenv` lacks the `axon_hooks` module that
    run_bass_kernel_spmd imports when NTFF tracing is requested (BASS_TRACE=1).
    Install an equivalent shim so profiling works instead of crashing."""
    try:
        import antenv.axon_hooks  # noqa: F401
        return
    except ImportError:
        pass
    import sys
    import types
    try:
        import antenv
    except ImportError:
        return
    mod = types.ModuleType("antenv.axon_hooks")
    holder = {"hook": None}
    mod.set_axon_ntff_profile_hook = lambda h: holder.__setitem__("hook", h)
    mod.get_axon_ntff_profile_hook = lambda: holder["hook"]
    sys.modules["antenv.axon_hooks"] = mod
    antenv.axon_hooks = mod
    so_path = "/opt/axon/libaxon_pjrt.so"
    try:
        import os
        if os.path.exists(so_path):
            from trn_agent_boot.trn_boot import _ntff_profile_via_ctypes
            hook = _ntff_profile_via_ctypes(so_path)
            if hook is not None:
                mod.set_axon_ntff_profile_hook(hook)
    except Exception:
        pass  # hook stays None; bass_utils logs a warning and skips tracing


def _chunk_sizes(m: int) -> list[int]:
    """Split m into chunks <= 512 with the LAST chunk ~TAILN (tail drain)."""
    if m <= MAXN:
        return [m]
    rest = m - TAILN
    n = -(-rest // MAXN)
    base, rem = divmod(rest, n)
    return [base + 1] * rem + [base] * (n - rem) + [TAILN]


def _build(m_pad: int):
    nc = bacc.Bacc("TRN2", target_bir_lowering=False, debug=False,
                   num_devices=N_CORES)

    FI2 = I2 // P  # 18 f-tiles of the Wi output
    xe = nc.dram_tensor("xe", [H, m_pad], BF16, kind="ExternalInput").ap()
    wi = nc.dram_tensor("wi", [H, I2], BF16, kind="ExternalInput").ap()
    # expert Wo arrives fp8e4 (host pre-scaled x16, partition-major): the
    # expert second matmul runs in fp8 DoubleRow (2 contraction rows/cell,
    # ~1.4x PE throughput); host divides the combine weights by 16
    wo8 = nc.dram_tensor("wo8", [P, KI, H], FP8, kind="ExternalInput").ap()
    xs = nc.dram_tensor("xs", [H, TS], BF16, kind="ExternalInput").ap()
    # swi arrives host-pre-tiled: swi[ft, p, kt*P+c] = shared_Wi.T[kt*P+p, ft*P+c]
    # so each 128-wide f-tile is one contiguous DMA, loadable in the exact
    # order the PE consumes them during the (DMA-bound) kernel lead-in.
    swi = nc.dram_tensor("swi", [FI2, P, H], BF16, kind="ExternalInput").ap()
    swo = nc.dram_tensor("swo", [I, H], BF16, kind="ExternalInput").ap()
    # outputs are chunk-major flat [P, KH * m]: column cb + ht*sz + j holds
    # feature (ht*128+p) of token (off+j) for the chunk at token-offset off
    # (cb = off*KH). Each half-chunk output DMA is then one fully-contiguous
    # per-partition run -> 1536B descriptors instead of 512B ones (the
    # [H, m] row-major layout capped descriptor runs at sz*2B and the tail
    # DMA drained at ~170 GB/s).
    ye = nc.dram_tensor("ye", [P, KH * m_pad], BF16, kind="ExternalOutput").ap()
    ys = nc.dram_tensor("ys", [P, KH * TS], BF16, kind="ExternalOutput").ap()

    with ExitStack() as ctx:
        tc = ctx.enter_context(tile.TileContext(nc))
        wpool = ctx.enter_context(tc.tile_pool(name="weights", bufs=1))
        apool = ctx.enter_context(tc.tile_pool(name="act", bufs=3))
        spool = ctx.enter_context(tc.tile_pool(name="silu", bufs=4))
        ypool = ctx.enter_context(tc.tile_pool(name="y", bufs=3))
        # all 8 PSUM banks in one pool; the warm-up tile shares the "ps" tag
        # so its bank recycles into the working set after the lead-in
        psum = ctx.enter_context(tc.tile_pool(name="psum", bufs=8, space="PSUM"))

        # Warm-up matmuls on scratch data fill the otherwise-idle PE during
        # the DMA lead-in: the HAM clock gate sees a busy window and
        # un-throttles (4/8 -> 8/8) around when the real matmuls start.
        # memset on gpsimd (free at ~4.7us; the vector queue's own init would
        # delay it to ~8us as measured in the v1 trace).
        warm_sink = nc.dram_tensor("warm_sink", [P, MAXN], F32).ap()
        warm_sb = wpool.tile([P, MAXN], BF16, tag="warm", name="warm")
        nc.gpsimd.memset(warm_sb[:], 0.0)
        wps = psum.tile([P, MAXN], F32, tag="ps", name="wps")
        # 9 cold matmuls x ~427ns from ~7.6us bridge past when the first
        # real chain's data lands (~10.5us incl. semaphore latency): the
        # HAM un-throttle needs a fully-busy 3.4us window, so the warm-up
        # must merge gaplessly into the (DMA-paced) first real chain
        for i in range(9):
            nc.tensor.matmul(wps, lhsT=warm_sb[:, :P], rhs=warm_sb[:],
                             start=True, stop=True)
        warm_out = spool.tile([P, MAXN], F32, tag="silu", name="warm_out")
        nc.vector.tensor_copy(warm_out[:], wps)

        # All resident tensors (x and weights) are loaded as per-k-tile SBUF
        # tiles: dependency tracking is per tile, so a matmul only waits for
        # the one 128-row slice it reads, and compute starts as soon as the
        # first slices land instead of after the whole preload.
        def load_rows(eng, dram_ap, ktiles, tag, fsplit=1):
            # half-major emission order (h outer): all chain-A weight tiles
            # stream before the chain-B tiles, matching PE consumption
            src = dram_ap.rearrange("(o p) f -> p o f", p=P)
            fw = dram_ap.shape[1] // fsplit
            out = [[None] * fsplit for _ in range(ktiles)]
            for h in range(fsplit):
                for kt in range(ktiles):
                    t = wpool.tile([P, fw], BF16, tag=f"{tag}{kt}_{h}",
                                   name=f"{tag}{kt}_{h}")
                    eng.dma_start(t[:], src[:, kt, h * fw:(h + 1) * fw])
                    out[kt][h] = t
            return out

        # Lead-in: ONE priority-ordered sync-HWDGE stream. Parallel queues
        # only split the shared ~320 GB/s of HBM bandwidth (measured: xs on
        # the scalar queue trickled at 1.3us/tile next to this stream and
        # paced the whole first chain), so the first chain's working set
        # goes out front in exact consumption order:
        #   xs0, swiA0, swiB0, xs1..xs5, swiA1, swiB1, swiA2, ...
        xs_src = xs.rearrange("(o p) f -> p o f", p=P)
        xs_t = [None] * KH
        swi_f = [None] * FI2

        def load_xs(kt):
            tx = wpool.tile([P, TS], BF16, tag=f"xs{kt}", name=f"xs{kt}")
            nc.sync.dma_start(tx[:], xs_src[:, kt])
            xs_t[kt] = [tx]

        def load_swi(f, eng=None):
            t = wpool.tile([P, H], BF16, tag=f"swiF{f}", name=f"swiF{f}")
            (eng or nc.sync).dma_start(t[:], swi[f])
            swi_f[f] = t

        # first chain pair (F0, F9) rides the scalar queue in parallel with
        # the xs stream on sync: the first real chain's full working set is
        # resident ~1us before the warm-up ends, so the warm-up merges into
        # the chain with NO bubble (a bubble resets the HAM busy window and
        # costs ~1.6us of cold-clock time on unlucky window phases)
        load_swi(0, nc.scalar)
        load_swi(KI, nc.scalar)
        for kt in range(KH):
            load_xs(kt)
        for ft in range(1, KI):
            load_swi(ft)
            load_swi(KI + ft)

        named = {}
        named["swo"] = load_rows(nc.sync, swo, KI, "swo")
        named["xe"] = load_rows(nc.sync, xe, KH, "xe")
        named["wi"] = load_rows(nc.sync, wi, KH, "wi", fsplit=2)
        wo8_t = wpool.tile([P, KI, H], FP8, tag="wo8", name="wo8")
        nc.sync.dma_start(wo8_t[:], wo8)

        # warm-up PSUM drain parks in the sync queue's mid-kernel idle window
        # (it must exist so the warm matmuls have a live consumer, but must
        # not sit at the end of any queue where it would extend the tail)
        nc.sync.dma_start(warm_sink[:], warm_out[:])

        # accessors: (ft|ht, kt) -> lhsT AP; x: (kt) -> rhs tile
        sh = dict(
            x=lambda kt: xs_t[kt][0],
            wa=lambda ft, kt: swi_f[ft][:, kt * P:(kt + 1) * P],
            wb=lambda ft, kt: swi_f[KI + ft][:, kt * P:(kt + 1) * P],
            wo=lambda ht, kt: named["swo"][kt][0][:, ht * P:(ht + 1) * P],
        )
        ex = dict(
            x=lambda kt: named["xe"][kt][0],
            wa=lambda ft, kt: named["wi"][kt][0][:, ft * P:(ft + 1) * P],
            wb=lambda ft, kt: named["wi"][kt][1][:, ft * P:(ft + 1) * P],
        )

        # (accessors, y_dram, chunk_off, chunk_sz, silu_on_first)
        chunks = []
        for acc, yd, m, sfirst in ((sh, ys, TS, True), (ex, ye, m_pad, False)):
            off = 0
            for sz in _chunk_sizes(m):
                chunks.append((acc, yd, off, sz, sfirst))
                off += sz

        def emit_wi(c):
            acc, yd, off, sz, sfirst = chunks[c]
            # shared chunk keeps bf16 activations; expert chunks store them
            # as fp8e4 so the Wo pass can run in DoubleRow mode
            adt, atag = (BF16, "act") if sfirst else (FP8, "act8")
            act = apool.tile([P, KI, MAXN], adt, tag=atag, name=atag)[:, :, :sz]
            for ft in range(KI):
                ps_a = psum.tile([P, MAXN], F32, tag="ps", name="ps_a")[:, :sz]
                for kt in range(KH):
                    nc.tensor.matmul(ps_a, lhsT=acc["wa"](ft, kt),
                                     rhs=acc["x"](kt)[:, off:off + sz],
                                     start=(kt == 0), stop=(kt == KH - 1))
                ps_b = psum.tile([P, MAXN], F32, tag="ps", name="ps_b")[:, :sz]
                for kt in range(KH):
                    nc.tensor.matmul(ps_b, lhsT=acc["wb"](ft, kt),
                                     rhs=acc["x"](kt)[:, off:off + sz],
                                     start=(kt == 0), stop=(kt == KH - 1))
                sl = spool.tile([P, MAXN], F32, tag="silu", name="sl")[:, :sz]
                ps_s, ps_m = (ps_a, ps_b) if sfirst else (ps_b, ps_a)
                if USE_SILU:
                    # act = silu(s) * m: one ACT op + one DVE mul; PSUM banks
                    # are freed one op earlier than the sigmoid+2-mul form
                    nc.scalar.activation(sl, ps_s,
                                         mybir.ActivationFunctionType.Silu)
                    nc.vector.tensor_mul(act[:, ft, :], sl, ps_m)
                else:
                    # CoreSim fallback: silu(s) = s * sigmoid(s)
                    tmp = spool.tile([P, MAXN], F32, tag="silu2",
                                     name="tmp")[:, :sz]
                    nc.scalar.activation(sl, ps_s,
                                         mybir.ActivationFunctionType.Sigmoid)
                    nc.vector.tensor_mul(tmp, sl, ps_s)
                    nc.vector.tensor_mul(act[:, ft, :], tmp, ps_m)
            return act

        def emit_wo(c, act, last=False):
            acc, yd, off, sz, sfirst = chunks[c]
            cb = off * KH
            # last chunk: first DMA carries h0-h4 so the tail's critical
            # path is one small copy + one 1-h-tile DMA; its copies run on
            # the (idle) DVE whose PSUM-read copy is ~2x faster than ACT's
            split = (KH - 1 if last else KH // 2) * sz
            yt = ypool.tile([P, KH * MAXN], BF16, tag="y", name="yt")
            # copy (f32 PSUM -> bf16 SBUF) on the Scalar engine (GpSimd has
            # no PSUM access) so DVE-mul throughput isn't what frees PSUM
            # banks; outputs leave as two merged 3-h-tile DMAs so the tail
            # is one small copy + one small DMA
            dma_eng = nc.scalar if last else nc.gpsimd
            for ht in range(KH):
                ps_y = psum.tile([P, MAXN], F32, tag="ps", name="ps_y")[:, :sz]
                hs = slice(ht * P, (ht + 1) * P)
                if sfirst:
                    # shared expert: bf16 9-step k-chain
                    for kt in range(KI):
                        nc.tensor.matmul(ps_y, lhsT=acc["wo"](ht, kt),
                                         rhs=act[:, kt, :],
                                         start=(kt == 0), stop=(kt == KI - 1))
                else:
                    # expert: fp8 DoubleRow over 4 k-tile pairs (256-row
                    # contraction each) + one plain fp8 matmul for k-tile 8
                    for g in range(4):
                        nc.tensor.matmul(
                            ps_y, lhsT=wo8_t[:, 2 * g:2 * g + 2, hs],
                            rhs=act[:, 2 * g:2 * g + 2, :],
                            start=(g == 0), stop=False,
                            perf_mode=mybir.MatmulPerfMode.DoubleRow)
                    nc.tensor.matmul(ps_y, lhsT=wo8_t[:, KI - 1, hs],
                                     rhs=act[:, KI - 1, :],
                                     start=False, stop=True)
                if last:
                    nc.vector.tensor_copy(yt[:, ht * sz:(ht + 1) * sz], ps_y)
                else:
                    nc.scalar.copy(yt[:, ht * sz:(ht + 1) * sz], ps_y)
                if (ht + 1) * sz == split:
                    dma_eng.dma_start(yd[:, cb:cb + split], yt[:, :split])
            dma_eng.dma_start(yd[:, cb + split:cb + KH * sz],
                              yt[:, split:KH * sz])

        # software pipeline: after Wo0 (long-resident shared weights),
        # Wi(c+1) is emitted before Wo(c) so the PE always has independent
        # matmul work while ACT/DVE finish chunk c's SwiGLU.
        n = len(chunks)
        acts = [None] * n
        acts[0] = emit_wi(0)
        if n == 1:
            emit_wo(0, acts[0], last=True)
        else:
            emit_wo(0, acts[0])
            acts[1] = emit_wi(1)
            for c in range(2, n):
                acts[c] = emit_wi(c)
                emit_wo(c - 1, acts[c - 1])
            emit_wo(n - 1, acts[n - 1], last=True)

    nc.compile()
    return nc


def _tile_swi(swiT):
    """(H, 2I) -> (18, P, H): f-tile-major contiguous layout for the device."""
    FI2 = I2 // P
    return np.ascontiguousarray(
        swiT.reshape(KH, P, FI2, P).transpose(2, 1, 0, 3).reshape(FI2, P, H))


def _route(x, gate_w, correction_bias):
    logits = 1.0 / (1.0 + np.exp(-(x @ gate_w.T), dtype=np.float32))  # (T, E)
    sel = logits + correction_bias[None, :]
    order = np.argsort(-sel, axis=1, kind="stable")[:, :TOPK]  # ties -> low index
    w = np.take_along_axis(logits, order, axis=1)
    w = (w / w.sum(axis=1, keepdims=True)).astype(np.float32)
    return order, w


def kernel(**inputs) -> np.ndarray:
    x = np.asarray(inputs["x"], np.float32)
    gate_w = np.asarray(inputs["gate_w"], np.float32)
    bias = np.asarray(inputs["correction_bias"], np.float32)
    Wi = np.asarray(inputs["Wi"], np.float32)
    Wo = np.asarray(inputs["Wo"], np.float32)
    shared_Wi = np.asarray(inputs["shared_Wi"], np.float32)
    shared_Wo = np.asarray(inputs["shared_Wo"], np.float32)

    order, w = _route(x, gate_w, bias)

    idx_per_e, cw_per_e = [], []
    for e in range(E):
        mask = order == e  # (T, K)
        tok = mask.any(axis=1)
        rows = np.nonzero(tok)[0]
        kpos = np.argmax(mask[rows], axis=1)
        idx_per_e.append(rows)
        cw_per_e.append(w[rows, kpos].astype(np.float32))

    mx = max(len(r) for r in idx_per_e)
    m_pad = max(64, mx + (mx & 1))  # exact capacity, kept even for alignment

    bf = ml_dtypes.bfloat16
    f8 = ml_dtypes.float8_e4m3fn
    xT = np.ascontiguousarray(x.T)  # (H, T) f32
    swiT = _tile_swi(shared_Wi.T.astype(bf))             # (18, P, H)
    swoT = np.ascontiguousarray(shared_Wo.T).astype(bf)  # (I, H)

    in_maps = []
    for c in range(N_CORES):
        rows = idx_per_e[c]
        xe = np.zeros((H, m_pad), bf)
        xe[:, :len(rows)] = xT[:, rows].astype(bf)
        # expert Wo: x16 (into fp8e4's well-conditioned range; the host
        # combine divides the weights by 16), partition-major [P, KI, H]
        wo8 = np.ascontiguousarray(
            (Wo[c] * 16.0).reshape(KI, P, H).transpose(1, 0, 2)).astype(f8)
        in_maps.append({
            "xe": xe,
            "wi": Wi[c].astype(bf),                      # (H, 2I)
            "wo8": wo8,                                  # (P, KI, H) fp8
            "xs": np.ascontiguousarray(
                xT[:, c * TS:(c + 1) * TS]).astype(bf),  # (H, TS)
            "swi": swiT,
            "swo": swoT,
        })

    if m_pad not in _BUILD_CACHE:
        _BUILD_CACHE[m_pad] = _build(m_pad)
    nc = _BUILD_CACHE[m_pad]

    _ensure_axon_ntff_hook()
    res = run_bass_kernel_spmd(nc, in_maps, list(range(N_CORES)))
    global LAST_RESULTS
    LAST_RESULTS = res

    def _unflatten(flat, m):
        # [P, KH*m] chunk-major flat -> (m, H): chunk at token-offset off
        # occupies columns [off*KH, (off+sz)*KH); within it, column
        # ht*sz + j holds feature ht*128+p of token off+j
        y = np.empty((m, H), np.float32)
        off = 0
        for sz in _chunk_sizes(m):
            blk = flat[:, off * KH:(off + sz) * KH].reshape(P, KH, sz)
            y[off:off + sz] = blk.transpose(1, 0, 2).reshape(H, sz).T
            off += sz
        return y

    out = np.zeros((T, H), np.float32)
    for c in range(N_CORES):
        r = res.results[c]
        ys_f = _unflatten(np.asarray(r["ys"]).astype(np.float32), TS)
        out[c * TS:(c + 1) * TS] += ys_f
        rows = idx_per_e[c]
        if len(rows):
            ye_f = _unflatten(np.asarray(r["ye"]).astype(np.float32), m_pad)
            out[rows] += ye_f[:len(rows)] * (cw_per_e[c][:, None] / 16.0)
    return out


# revision 32
# speedup vs baseline: 1.0104x; 1.0104x over previous
"""MoE layer (8 experts, top-2 sigmoid routing, SwiGLU experts + shared expert)
on 8 TRN2 NeuronCores.

Strategy (expert-parallel, host-side token dispatch):
  - Router (sigmoid(x @ gate_w.T), top-2, weight normalization) is tiny
    (~50 MFLOP) and runs on the host; it determines the all-to-all dispatch.
  - Core c owns expert c: it gets the tokens routed to expert c (gathered and
    zero-padded to a common capacity M_pad) plus expert c's Wi/Wo.
  - The shared expert is data-parallel: core c also processes tokens
    [c*512, (c+1)*512) with the (replicated) shared weights.
  - Device kernel computes the SwiGLU MLP passes with fp32 PSUM
    accumulation, feature-major layout (features on partitions, tokens on
    the free dim) so no on-device transposes are needed. The Wi pass and
    the shared expert run in bf16; the expert Wo pass runs in fp8 e4m3
    with DoubleRow perf mode (2 contraction rows per PE cell, ~1.4x PE
    throughput; rel-err ~1.7e-2 vs the 2e-2 gate, validated by numpy sim).
  - Host combine: out[t] = shared_out[t] + sum_e cw[e,t] * expert_out[e][t]
    (the combine weights, divided by the fp8 weight scale 16, are applied
    on the host during the scatter-add).

Schedule (v3):
  - HBM bandwidth (~320 GB/s) is shared by all DMA queues, so the input
    set streams on ONE priority-ordered sync-HWDGE queue (swi in exact PE
    consumption order, then swo, xe, wi, wo); only the 6 small xs tiles
    ride the scalar queue in parallel so the very first matmul chain has
    its rhs by ~6us (v2's three-way parallel streams split the bandwidth
    and starved the latency-critical shared-expert stream).
  - 6 warm-up matmuls (dep: a gpsimd memset available at ~4.7us) keep the
    PE busy from ~5us so the HAM clock gate un-throttles (4/8 -> 8/8)
    around the time real data lands.
  - Scalar runs ONLY Silu (one ACT table load); PSUM->SBUF output copies
    run on the otherwise-idle GpSimd engine with an f32->bf16 cast.
  - Outputs are bf16 (halves output DMA bytes); each chunk's 6 h-tiles
    leave as two merged 3-h-tile DMAs on gpsimd SWDGE, except the last
    chunk's which ride the (idle, lower-latency) scalar HWDGE queue.
  - Expert chunks are sized so the LAST chunk is ~256 tokens: the tail
    after the final matmul is one small copy + one small DMA + receipt.
  - Emit order Wi0 Wo0 Wi1 Wi2 Wo1 Wi3 Wo2 Wo3: Wo0 uses the
    long-resident shared weights while the expert stream is still in
    flight; afterwards Wi(c+1) precedes Wo(c) so the PE always has
    independent matmul work while ACT/DVE finish a chunk's SwiGLU.
"""

from contextlib import ExitStack

import ml_dtypes
import numpy as np

import concourse.tile as tile
from concourse import bacc, mybir
from concourse.bass_utils import run_bass_kernel_spmd

E, TOPK, H, I = 8, 2, 768, 1152
I2 = 2 * I
T = 4096
N_CORES = 8
TS = T // N_CORES  # shared-expert tokens per core
P = 128
KH = H // P    # 6 contraction tiles over H
KI = I // P    # 9 contraction tiles over I
BF16 = mybir.dt.bfloat16
F32 = mybir.dt.float32
FP8 = mybir.dt.float8e4  # e4m3
MAXN = 512     # max tokens per matmul chunk (one fp32 PSUM bank)
TAILN = 256    # target size of the LAST expert chunk (short drain)

_BUILD_CACHE: dict = {}
LAST_RESULTS = None  # BassKernelResults of the most recent device run
USE_SILU = True  # native ACT Silu on HW; set False for CoreSim (not implemented there)


def _ensure_axon_ntff_hook():
    """This image's `antenv` lacks the `axon_hooks` module that
    run_bass_kernel_spmd imports when NTFF tracing is requested (BASS_TRACE=1).
    Install an equivalent shim so profiling works instead of crashing."""
    try:
        import antenv.axon_hooks  # noqa: F401
        return
    except ImportError:
        pass
    import sys
    import types
    try:
        import antenv
    except ImportError:
        return
    mod = types.ModuleType("antenv.axon_hooks")
    holder = {"hook": None}
    mod.set_axon_ntff_profile_hook = lambda h: holder.__setitem__("hook", h)
    mod.get_axon_ntff_profile_hook = lambda: holder["hook"]
    sys.modules["antenv.axon_hooks"] = mod
    antenv.axon_hooks = mod
    so_path = "/opt/axon/libaxon_pjrt.so"
    try:
        import os
        if os.path.exists(so_path):
            from trn_agent_boot.trn_boot import _ntff_profile_via_ctypes
            hook = _ntff_profile_via_ctypes(so_path)
            if hook is not None:
                mod.set_axon_ntff_profile_hook(hook)
    except Exception:
        pass  # hook stays None; bass_utils logs a warning and skips tracing


def _chunk_sizes(m: int) -> list[int]:
    """Split m into chunks <= 512 with the LAST chunk ~TAILN (tail drain)."""
    if m <= MAXN:
        return [m]
    rest = m - TAILN
    n = -(-rest // MAXN)
    base, rem = divmod(rest, n)
    return [base + 1] * rem + [base] * (n - rem) + [TAILN]


def _build(m_pad: int):
    nc = bacc.Bacc("TRN2", target_bir_lowering=False, debug=False,
                   num_devices=N_CORES)

    FI2 = I2 // P  # 18 f-tiles of the Wi output
    xe = nc.dram_tensor("xe", [H, m_pad], BF16, kind="ExternalInput").ap()
    wi = nc.dram_tensor("wi", [H, I2], BF16, kind="ExternalInput").ap()
    # expert Wo arrives fp8e4 (host pre-scaled x16, partition-major): the
    # expert second matmul runs in fp8 DoubleRow (2 contraction rows/cell,
    # ~1.4x PE throughput); host divides the combine weights by 16
    wo8 = nc.dram_tensor("wo8", [P, KI, H], FP8, kind="ExternalInput").ap()
    xs = nc.dram_tensor("xs", [H, TS], BF16, kind="ExternalInput").ap()
    # swi arrives host-pre-tiled: swi[ft, p, kt*P+c] = shared_Wi.T[kt*P+p, ft*P+c]
    # so each 128-wide f-tile is one contiguous DMA, loadable in the exact
    # order the PE consumes them during the (DMA-bound) kernel lead-in.
    swi = nc.dram_tensor("swi", [FI2, P, H], BF16, kind="ExternalInput").ap()
    swo = nc.dram_tensor("swo", [I, H], BF16, kind="ExternalInput").ap()
    # outputs are chunk-major flat [P, KH * m]: column cb + ht*sz + j holds
    # feature (ht*128+p) of token (off+j) for the chunk at token-offset off
    # (cb = off*KH). Each half-chunk output DMA is then one fully-contiguous
    # per-partition run -> 1536B descriptors instead of 512B ones (the
    # [H, m] row-major layout capped descriptor runs at sz*2B and the tail
    # DMA drained at ~170 GB/s).
    ye = nc.dram_tensor("ye", [P, KH * m_pad], BF16, kind="ExternalOutput").ap()
    ys = nc.dram_tensor("ys", [P, KH * TS], BF16, kind="ExternalOutput").ap()

    with ExitStack() as ctx:
        tc = ctx.enter_context(tile.TileContext(nc))
        wpool = ctx.enter_context(tc.tile_pool(name="weights", bufs=1))
        apool = ctx.enter_context(tc.tile_pool(name="act", bufs=3))
        spool = ctx.enter_context(tc.tile_pool(name="silu", bufs=4))
        ypool = ctx.enter_context(tc.tile_pool(name="y", bufs=3))
        # all 8 PSUM banks in one pool; the warm-up tile shares the "ps" tag
        # so its bank recycles into the working set after the lead-in
        psum = ctx.enter_context(tc.tile_pool(name="psum", bufs=8, space="PSUM"))

        # Warm-up matmuls on scratch data fill the otherwise-idle PE during
        # the DMA lead-in: the HAM clock gate sees a busy window and
        # un-throttles (4/8 -> 8/8) around when the real matmuls start.
        # memset on gpsimd (free at ~4.7us; the vector queue's own init would
        # delay it to ~8us as measured in the v1 trace).
        warm_sink = nc.dram_tensor("warm_sink", [P, MAXN], F32).ap()
        warm_sb = wpool.tile([P, MAXN], BF16, tag="warm", name="warm")
        nc.gpsimd.memset(warm_sb[:], 0.0)
        wps = psum.tile([P, MAXN], F32, tag="ps", name="wps")
        # 9 cold matmuls x ~427ns from ~7.6us bridge past when the first
        # real chain's data lands (~10.5us incl. semaphore latency): the
        # HAM un-throttle needs a fully-busy 3.4us window, so the warm-up
        # must merge gaplessly into the (DMA-paced) first real chain
        for i in range(9):
            nc.tensor.matmul(wps, lhsT=warm_sb[:, :P], rhs=warm_sb[:],
                             start=True, stop=True)
        warm_out = spool.tile([P, MAXN], F32, tag="silu", name="warm_out")
        nc.vector.tensor_copy(warm_out[:], wps)

        # All resident tensors (x and weights) are loaded as per-k-tile SBUF
        # tiles: dependency tracking is per tile, so a matmul only waits for
        # the one 128-row slice it reads, and compute starts as soon as the
        # first slices land instead of after the whole preload.
        def load_rows(eng, dram_ap, ktiles, tag, fsplit=1):
            # half-major emission order (h outer): all chain-A weight tiles
            # stream before the chain-B tiles, matching PE consumption
            src = dram_ap.rearrange("(o p) f -> p o f", p=P)
            fw = dram_ap.shape[1] // fsplit
            out = [[None] * fsplit for _ in range(ktiles)]
            for h in range(fsplit):
                for kt in range(ktiles):
                    t = wpool.tile([P, fw], BF16, tag=f"{tag}{kt}_{h}",
                                   name=f"{tag}{kt}_{h}")
                    eng.dma_start(t[:], src[:, kt, h * fw:(h + 1) * fw])
                    out[kt][h] = t
            return out

        # Lead-in: ONE priority-ordered sync-HWDGE stream. Parallel queues
        # only split the shared ~320 GB/s of HBM bandwidth (measured: xs on
        # the scalar queue trickled at 1.3us/tile next to this stream and
        # paced the whole first chain), so the first chain's working set
        # goes out front in exact consumption order:
        #   xs0, swiA0, swiB0, xs1..xs5, swiA1, swiB1, swiA2, ...
        xs_src = xs.rearrange("(o p) f -> p o f", p=P)
        xs_t = [None] * KH
        swi_f = [None] * FI2

        def load_xs(kt):
            tx = wpool.tile([P, TS], BF16, tag=f"xs{kt}", name=f"xs{kt}")
            nc.sync.dma_start(tx[:], xs_src[:, kt])
            xs_t[kt] = [tx]

        def load_swi(f, eng=None):
            t = wpool.tile([P, H], BF16, tag=f"swiF{f}", name=f"swiF{f}")
            (eng or nc.sync).dma_start(t[:], swi[f])
            swi_f[f] = t

        # first chain pair (F0, F9) rides the scalar queue in parallel with
        # the xs stream on sync: the first real chain's full working set is
        # resident ~1us before the warm-up ends, so the warm-up merges into
        # the chain with NO bubble (a bubble resets the HAM busy window and
        # costs ~1.6us of cold-clock time on unlucky window phases)
        load_swi(0, nc.scalar)
        load_swi(KI, nc.scalar)
        for kt in range(KH):
            load_xs(kt)
        for ft in range(1, KI):
            load_swi(ft)
            load_swi(KI + ft)

        named = {}
        named["swo"] = load_rows(nc.sync, swo, KI, "swo")
        named["xe"] = load_rows(nc.sync, xe, KH, "xe")
        named["wi"] = load_rows(nc.sync, wi, KH, "wi", fsplit=2)
        wo8_t = wpool.tile([P, KI, H], FP8, tag="wo8", name="wo8")
        nc.sync.dma_start(wo8_t[:], wo8)

        # warm-up PSUM drain parks in the sync queue's mid-kernel idle window
        # (it must exist so the warm matmuls have a live consumer, but must
        # not sit at the end of any queue where it would extend the tail)
        nc.sync.dma_start(warm_sink[:], warm_out[:])

        # accessors: (ft|ht, kt) -> lhsT AP; x: (kt) -> rhs tile
        sh = dict(
            x=lambda kt: xs_t[kt][0],
            wa=lambda ft, kt: swi_f[ft][:, kt * P:(kt + 1) * P],
            wb=lambda ft, kt: swi_f[KI + ft][:, kt * P:(kt + 1) * P],
            wo=lambda ht, kt: named["swo"][kt][0][:, ht * P:(ht + 1) * P],
        )
        ex = dict(
            x=lambda kt: named["xe"][kt][0],
            wa=lambda ft, kt: named["wi"][kt][0][:, ft * P:(ft + 1) * P],
            wb=lambda ft, kt: named["wi"][kt][1][:, ft * P:(ft + 1) * P],
        )

        # (accessors, y_dram, chunk_off, chunk_sz, silu_on_first)
        chunks = []
        for acc, yd, m, sfirst in ((sh, ys, TS, True), (ex, ye, m_pad, False)):
            off = 0
            for sz in _chunk_sizes(m):
                chunks.append((acc, yd, off, sz, sfirst))
                off += sz

        def emit_wi(c):
            acc, yd, off, sz, sfirst = chunks[c]
            # shared chunk keeps bf16 activations; expert chunks store them
            # as fp8e4 so the Wo pass can run in DoubleRow mode
            adt, atag = (BF16, "act") if sfirst else (FP8, "act8")
            act = apool.tile([P, KI, MAXN], adt, tag=atag, name=atag)[:, :, :sz]
            for ft in range(KI):
                ps_a = psum.tile([P, MAXN], F32, tag="ps", name="ps_a")[:, :sz]
                for kt in range(KH):
                    nc.tensor.matmul(ps_a, lhsT=acc["wa"](ft, kt),
                                     rhs=acc["x"](kt)[:, off:off + sz],
                                     start=(kt == 0), stop=(kt == KH - 1))
                ps_b = psum.tile([P, MAXN], F32, tag="ps", name="ps_b")[:, :sz]
                for kt in range(KH):
                    nc.tensor.matmul(ps_b, lhsT=acc["wb"](ft, kt),
                                     rhs=acc["x"](kt)[:, off:off + sz],
                                     start=(kt == 0), stop=(kt == KH - 1))
                sl = spool.tile([P, MAXN], F32, tag="silu", name="sl")[:, :sz]
                ps_s, ps_m = (ps_a, ps_b) if sfirst else (ps_b, ps_a)
                if USE_SILU:
                    # act = silu(s) * m: one ACT op + one DVE mul; PSUM banks
                    # are freed one op earlier than the sigmoid+2-mul form
                    nc.scalar.activation(sl, ps_s,
                                         mybir.ActivationFunctionType.Silu)
                    nc.vector.tensor_mul(act[:, ft, :], sl, ps_m)
                else:
                    # CoreSim fallback: silu(s) = s * sigmoid(s)
                    tmp = spool.tile([P, MAXN], F32, tag="silu2",
                                     name="tmp")[:, :sz]
                    nc.scalar.activation(sl, ps_s,
                                         mybir.ActivationFunctionType.Sigmoid)
                    nc.vector.tensor_mul(tmp, sl, ps_s)
                    nc.vector.tensor_mul(act[:, ft, :], tmp, ps_m)
            return act

        def emit_wo(c, act, last=False):
            acc, yd, off, sz, sfirst = chunks[c]
            cb = off * KH
            # last chunk: first DMA carries h0-h4 so the tail's critical
            # path is one small copy + one 1-h-tile DMA; its copies run on
            # the (idle) DVE whose PSUM-read copy is ~2x faster than ACT's
            split = (KH - 1 if last else KH // 2) * sz
            yt = ypool.tile([P, KH * MAXN], BF16, tag="y", name="yt")
            # copy (f32 PSUM -> bf16 SBUF) on the Scalar engine (GpSimd has
            # no PSUM access) so DVE-mul throughput isn't what frees PSUM
            # banks; outputs leave as two merged 3-h-tile DMAs so the tail
            # is one small copy + one small DMA
            dma_eng = nc.scalar if last else nc.gpsimd
            for ht in range(KH):
                ps_y = psum.tile([P, MAXN], F32, tag="ps", name="ps_y")[:, :sz]
                hs = slice(ht * P, (ht + 1) * P)
                if sfirst:
                    # shared expert: bf16 9-step k-chain
                    for kt in range(KI):
                        nc.tensor.matmul(ps_y, lhsT=acc["wo"](ht, kt),
                                         rhs=act[:, kt, :],
                                         start=(kt == 0), stop=(kt == KI - 1))
                else:
                    # expert: fp8 DoubleRow over 4 k-tile pairs (256-row
                    # contraction each) + one plain fp8 matmul for k-tile 8
                    for g in range(4):
                        nc.tensor.matmul(
                            ps_y, lhsT=wo8_t[:, 2 * g:2 * g + 2, hs],
                            rhs=act[:, 2 * g:2 * g + 2, :],
                            start=(g == 0), stop=False,
                            perf_mode=mybir.MatmulPerfMode.DoubleRow)
                    nc.tensor.matmul(ps_y, lhsT=wo8_t[:, KI - 1, hs],
                                     rhs=act[:, KI - 1, :],
                                     start=False, stop=True)
                if last:
                    nc.vector.tensor_copy(yt[:, ht * sz:(ht + 1) * sz], ps_y)
                else:
                    nc.scalar.copy(yt[:, ht * sz:(ht + 1) * sz], ps_y)
                if (ht + 1) * sz == split:
                    dma_eng.dma_start(yd[:, cb:cb + split], yt[:, :split])
            dma_eng.dma_start(yd[:, cb + split:cb + KH * sz],
                              yt[:, split:KH * sz])

        # software pipeline: after Wo0 (long-resident shared weights),
        # Wi(c+1) is emitted before Wo(c) so the PE always has independent
        # matmul work while ACT/DVE finish chunk c's SwiGLU.
        n = len(chunks)
        acts = [None] * n
        acts[0] = emit_wi(0)
        if n == 1:
            emit_wo(0, acts[0], last=True)
        else:
            emit_wo(0, acts[0])
            acts[1] = emit_wi(1)
            for c in range(2, n):
                acts[c] = emit_wi(c)
                emit_wo(c - 1, acts[c - 1])
            emit_wo(n - 1, acts[n - 1], last=True)

    nc.compile()
    return nc


def _tile_swi(swiT):
    """(H, 2I) -> (18, P, H): f-tile-major contiguous layout for the device."""
    FI2 = I2 // P
    return np.ascontiguousarray(
        swiT.reshape(KH, P, FI2, P).transpose(2, 1, 0, 3).reshape(FI2, P, H))


def _route(x, gate_w, correction_bias):
    logits = 1.0 / (1.0 + np.exp(-(x @ gate_w.T), dtype=np.float32))  # (T, E)
    sel = logits + correction_bias[None, :]
    order = np.argsort(-sel, axis=1, kind="stable")[:, :TOPK]  # ties -> low index
    w = np.take_along_axis(logits, order, axis=1)
    w = (w / w.sum(axis=1, keepdims=True)).astype(np.float32)
    return order, w


def kernel(**inputs) -> np.ndarray:
    x = np.asarray(inputs["x"], np.float32)
    gate_w = np.asarray(inputs["gate_w"], np.float32)
    bias = np.asarray(inputs["correction_bias"], np.float32)
    Wi = np.asarray(inputs["Wi"], np.float32)
    Wo = np.asarray(inputs["Wo"], np.float32)
    shared_Wi = np.asarray(inputs["shared_Wi"], np.float32)
    shared_Wo = np.asarray(inputs["shared_Wo"], np.float32)

    order, w = _route(x, gate_w, bias)

    idx_per_e, cw_per_e = [], []
    for e in range(E):
        mask = order == e  # (T, K)
        tok = mask.any(axis=1)
        rows = np.nonzero(tok)[0]
        kpos = np.argmax(mask[rows], axis=1)
        idx_per_e.append(rows)
        cw_per_e.append(w[rows, kpos].astype(np.float32))

    mx = max(len(r) for r in idx_per_e)
    m_pad = max(64, mx + (mx & 1))  # exact capacity, kept even for alignment

    bf = ml_dtypes.bfloat16
    f8 = ml_dtypes.float8_e4m3fn
    xT = np.ascontiguousarray(x.T)  # (H, T) f32
    swiT = _tile_swi(shared_Wi.T.astype(bf))             # (18, P, H)
    swoT = np.ascontiguousarray(shared_Wo.T).astype(bf)  # (I, H)

    in_maps = []
    for c in range(N_CORES):
        rows = idx_per_e[c]
        xe = np.zeros((H, m_pad), bf)
        xe[:, :len(rows)] = xT[:, rows].astype(bf)
        # expert Wo: x16 (into fp8e4's well-conditioned range; the host
        # combine divides the weights by 16), partition-major [P, KI, H]
        wo8 = np.ascontiguousarray(
            (Wo[c] * 16.0).reshape(KI, P, H).transpose(1, 0, 2)).astype(f8)
        in_maps.append({
            "xe": xe,
            "wi": Wi[c].astype(bf),                      # (H, 2I)
            "wo8": wo8,                                  # (P, KI, H) fp8
            "xs": np.ascontiguousarray(
                xT[:, c * TS:(c + 1) * TS]).astype(bf),  # (H, TS)
            "swi": swiT,
            "swo": swoT,
        })

    if m_pad not in _BUILD_CACHE:
        _BUILD_CACHE[m_pad] = _build(m_pad)
    nc = _BUILD_CACHE[m_pad]

    _ensure_axon_ntff_hook()
    res = run_bass_kernel_spmd(nc, in_maps, list(range(N_CORES)))
    global LAST_RESULTS
    LAST_RESULTS = res

    def _unflatten(flat, m):
        # [P, KH*m] chunk-major flat -> (m, H): chunk at token-offset off
        # occupies columns [off*KH, (off+sz)*KH); within it, column
        # ht*sz + j holds feature ht*128+p of token off+j
        y = np.empty((m, H), np.float32)
        off = 0
        for sz in _chunk_sizes(m):
            blk = flat[:, off * KH:(off + sz) * KH].reshape(P, KH, sz)
            y[off:off + sz] = blk.transpose(1, 0, 2).reshape(H, sz).T
            off += sz
        return y

    out = np.zeros((T, H), np.float32)
    for c in range(N_CORES):
        r = res.results[c]
        ys_f = _unflatten(np.asarray(r["ys"]).astype(np.float32), TS)
        out[c * TS:(c + 1) * TS] += ys_f
        rows = idx_per_e[c]
        if len(rows):
            ye_f = _unflatten(np.asarray(r["ye"]).astype(np.float32), m_pad)
            out[rows] += ye_f[:len(rows)] * (cw_per_e[c][:, None] / 16.0)
    return out


# revision 33
# speedup vs baseline: 1.0112x; 1.0008x over previous
"""MoE layer (8 experts, top-2 sigmoid routing, SwiGLU experts + shared expert)
on 8 TRN2 NeuronCores.

Strategy (expert-parallel, host-side token dispatch):
  - Router (sigmoid(x @ gate_w.T), top-2, weight normalization) is tiny
    (~50 MFLOP) and runs on the host; it determines the all-to-all dispatch.
  - Core c owns expert c: it gets the tokens routed to expert c (gathered and
    zero-padded to a common capacity M_pad) plus expert c's Wi/Wo.
  - The shared expert is data-parallel: core c also processes tokens
    [c*512, (c+1)*512) with the (replicated) shared weights.
  - Device kernel computes the SwiGLU MLP passes with fp32 PSUM
    accumulation, feature-major layout (features on partitions, tokens on
    the free dim) so no on-device transposes are needed. The Wi pass and
    the shared expert run in bf16; the expert Wo pass runs in fp8 e4m3
    with DoubleRow perf mode (2 contraction rows per PE cell, ~1.4x PE
    throughput; rel-err ~1.7e-2 vs the 2e-2 gate, validated by numpy sim).
  - Host combine: out[t] = shared_out[t] + sum_e cw[e,t] * expert_out[e][t]
    (the combine weights, divided by the fp8 weight scale 16, are applied
    on the host during the scatter-add).

Schedule (v3):
  - HBM bandwidth (~320 GB/s) is shared by all DMA queues, so the input
    set streams on ONE priority-ordered sync-HWDGE queue (swi in exact PE
    consumption order, then swo, xe, wi, wo); only the 6 small xs tiles
    ride the scalar queue in parallel so the very first matmul chain has
    its rhs by ~6us (v2's three-way parallel streams split the bandwidth
    and starved the latency-critical shared-expert stream).
  - 6 warm-up matmuls (dep: a gpsimd memset available at ~4.7us) keep the
    PE busy from ~5us so the HAM clock gate un-throttles (4/8 -> 8/8)
    around the time real data lands.
  - Scalar runs ONLY Silu (one ACT table load); PSUM->SBUF output copies
    run on the otherwise-idle GpSimd engine with an f32->bf16 cast.
  - Outputs are bf16 (halves output DMA bytes); each chunk's 6 h-tiles
    leave as two merged 3-h-tile DMAs on gpsimd SWDGE, except the last
    chunk's which ride the (idle, lower-latency) scalar HWDGE queue.
  - Expert chunks are sized so the LAST chunk is ~256 tokens: the tail
    after the final matmul is one small copy + one small DMA + receipt.
  - Emit order Wi0 Wo0 Wi1 Wi2 Wo1 Wi3 Wo2 Wo3: Wo0 uses the
    long-resident shared weights while the expert stream is still in
    flight; afterwards Wi(c+1) precedes Wo(c) so the PE always has
    independent matmul work while ACT/DVE finish a chunk's SwiGLU.
"""

from contextlib import ExitStack

import ml_dtypes
import numpy as np

import concourse.tile as tile
from concourse import bacc, mybir
from concourse.bass_utils import run_bass_kernel_spmd

E, TOPK, H, I = 8, 2, 768, 1152
I2 = 2 * I
T = 4096
N_CORES = 8
TS = T // N_CORES  # shared-expert tokens per core
P = 128
KH = H // P    # 6 contraction tiles over H
KI = I // P    # 9 contraction tiles over I
BF16 = mybir.dt.bfloat16
F32 = mybir.dt.float32
FP8 = mybir.dt.float8e4  # e4m3
MAXN = 512     # max tokens per matmul chunk (one fp32 PSUM bank)
TAILN = 256    # target size of the LAST expert chunk (short drain)

_BUILD_CACHE: dict = {}
LAST_RESULTS = None  # BassKernelResults of the most recent device run
USE_SILU = True  # native ACT Silu on HW; set False for CoreSim (not implemented there)


def _ensure_axon_ntff_hook():
    """This image's `antenv` lacks the `axon_hooks` module that
    run_bass_kernel_spmd imports when NTFF tracing is requested (BASS_TRACE=1).
    Install an equivalent shim so profiling works instead of crashing."""
    try:
        import antenv.axon_hooks  # noqa: F401
        return
    except ImportError:
        pass
    import sys
    import types
    try:
        import antenv
    except ImportError:
        return
    mod = types.ModuleType("antenv.axon_hooks")
    holder = {"hook": None}
    mod.set_axon_ntff_profile_hook = lambda h: holder.__setitem__("hook", h)
    mod.get_axon_ntff_profile_hook = lambda: holder["hook"]
    sys.modules["antenv.axon_hooks"] = mod
    antenv.axon_hooks = mod
    so_path = "/opt/axon/libaxon_pjrt.so"
    try:
        import os
        if os.path.exists(so_path):
            from trn_agent_boot.trn_boot import _ntff_profile_via_ctypes
            hook = _ntff_profile_via_ctypes(so_path)
            if hook is not None:
                mod.set_axon_ntff_profile_hook(hook)
    except Exception:
        pass  # hook stays None; bass_utils logs a warning and skips tracing


def _chunk_sizes(m: int) -> list[int]:
    """Split m into chunks <= 512 with the LAST chunk ~TAILN (tail drain)."""
    if m <= MAXN:
        return [m]
    rest = m - TAILN
    n = -(-rest // MAXN)
    base, rem = divmod(rest, n)
    return [base + 1] * rem + [base] * (n - rem) + [TAILN]


def _build(m_pad: int):
    nc = bacc.Bacc("TRN2", target_bir_lowering=False, debug=False,
                   num_devices=N_CORES)

    FI2 = I2 // P  # 18 f-tiles of the Wi output
    xe = nc.dram_tensor("xe", [H, m_pad], BF16, kind="ExternalInput").ap()
    wi = nc.dram_tensor("wi", [H, I2], BF16, kind="ExternalInput").ap()
    # expert Wo arrives fp8e4 (host pre-scaled x16, partition-major): the
    # expert second matmul runs in fp8 DoubleRow (2 contraction rows/cell,
    # ~1.4x PE throughput); host divides the combine weights by 16
    wo8 = nc.dram_tensor("wo8", [P, KI, H], FP8, kind="ExternalInput").ap()
    xs = nc.dram_tensor("xs", [H, TS], BF16, kind="ExternalInput").ap()
    # swi arrives host-pre-tiled: swi[ft, p, kt*P+c] = shared_Wi.T[kt*P+p, ft*P+c]
    # so each 128-wide f-tile is one contiguous DMA, loadable in the exact
    # order the PE consumes them during the (DMA-bound) kernel lead-in.
    swi = nc.dram_tensor("swi", [FI2, P, H], BF16, kind="ExternalInput").ap()
    swo = nc.dram_tensor("swo", [I, H], BF16, kind="ExternalInput").ap()
    # outputs are chunk-major flat [P, KH * m]: column cb + ht*sz + j holds
    # feature (ht*128+p) of token (off+j) for the chunk at token-offset off
    # (cb = off*KH). Each half-chunk output DMA is then one fully-contiguous
    # per-partition run -> 1536B descriptors instead of 512B ones (the
    # [H, m] row-major layout capped descriptor runs at sz*2B and the tail
    # DMA drained at ~170 GB/s).
    ye = nc.dram_tensor("ye", [P, KH * m_pad], BF16, kind="ExternalOutput").ap()
    ys = nc.dram_tensor("ys", [P, KH * TS], BF16, kind="ExternalOutput").ap()

    with ExitStack() as ctx:
        tc = ctx.enter_context(tile.TileContext(nc))
        wpool = ctx.enter_context(tc.tile_pool(name="weights", bufs=1))
        apool = ctx.enter_context(tc.tile_pool(name="act", bufs=3))
        spool = ctx.enter_context(tc.tile_pool(name="silu", bufs=4))
        ypool = ctx.enter_context(tc.tile_pool(name="y", bufs=3))
        # all 8 PSUM banks in one pool; the warm-up tile shares the "ps" tag
        # so its bank recycles into the working set after the lead-in
        psum = ctx.enter_context(tc.tile_pool(name="psum", bufs=8, space="PSUM"))

        # Warm-up matmuls on scratch data fill the otherwise-idle PE during
        # the DMA lead-in: the HAM clock gate sees a busy window and
        # un-throttles (4/8 -> 8/8) around when the real matmuls start.
        # memset on gpsimd (free at ~4.7us; the vector queue's own init would
        # delay it to ~8us as measured in the v1 trace).
        warm_sink = nc.dram_tensor("warm_sink", [P, MAXN], F32).ap()
        warm_sb = wpool.tile([P, MAXN], BF16, tag="warm", name="warm")
        nc.gpsimd.memset(warm_sb[:], 0.0)
        wps = psum.tile([P, MAXN], F32, tag="ps", name="wps")
        # 9 cold matmuls x ~427ns from ~7.6us bridge past when the first
        # real chain's data lands (~10.5us incl. semaphore latency): the
        # HAM un-throttle needs a fully-busy 3.4us window, so the warm-up
        # must merge gaplessly into the (DMA-paced) first real chain
        for i in range(9):
            nc.tensor.matmul(wps, lhsT=warm_sb[:, :P], rhs=warm_sb[:],
                             start=True, stop=True)
        warm_out = spool.tile([P, MAXN], F32, tag="silu", name="warm_out")
        nc.vector.tensor_copy(warm_out[:], wps)

        # All resident tensors (x and weights) are loaded as per-k-tile SBUF
        # tiles: dependency tracking is per tile, so a matmul only waits for
        # the one 128-row slice it reads, and compute starts as soon as the
        # first slices land instead of after the whole preload.
        def load_rows(eng, dram_ap, ktiles, tag, fsplit=1):
            # half-major emission order (h outer): all chain-A weight tiles
            # stream before the chain-B tiles, matching PE consumption
            src = dram_ap.rearrange("(o p) f -> p o f", p=P)
            fw = dram_ap.shape[1] // fsplit
            out = [[None] * fsplit for _ in range(ktiles)]
            for h in range(fsplit):
                for kt in range(ktiles):
                    t = wpool.tile([P, fw], BF16, tag=f"{tag}{kt}_{h}",
                                   name=f"{tag}{kt}_{h}")
                    eng.dma_start(t[:], src[:, kt, h * fw:(h + 1) * fw])
                    out[kt][h] = t
            return out

        # Lead-in: ONE priority-ordered sync-HWDGE stream. Parallel queues
        # only split the shared ~320 GB/s of HBM bandwidth (measured: xs on
        # the scalar queue trickled at 1.3us/tile next to this stream and
        # paced the whole first chain), so the first chain's working set
        # goes out front in exact consumption order:
        #   xs0, swiA0, swiB0, xs1..xs5, swiA1, swiB1, swiA2, ...
        xs_src = xs.rearrange("(o p) f -> p o f", p=P)
        xs_t = [None] * KH
        swi_f = [None] * FI2

        def load_xs(kt):
            tx = wpool.tile([P, TS], BF16, tag=f"xs{kt}", name=f"xs{kt}")
            nc.sync.dma_start(tx[:], xs_src[:, kt])
            xs_t[kt] = [tx]

        def load_swi(f, eng=None):
            t = wpool.tile([P, H], BF16, tag=f"swiF{f}", name=f"swiF{f}")
            (eng or nc.sync).dma_start(t[:], swi[f])
            swi_f[f] = t

        # first chain pair (F0, F9) rides the scalar queue in parallel with
        # the xs stream on sync: the first real chain's full working set is
        # resident ~1us before the warm-up ends, so the warm-up merges into
        # the chain with NO bubble (a bubble resets the HAM busy window and
        # costs ~1.6us of cold-clock time on unlucky window phases)
        load_swi(0, nc.scalar)
        load_swi(KI, nc.scalar)
        # F1/F10 slot between xs2 and xs3: they land ~2.5us before the f1
        # chain pair needs them (measured 484ns PE gap when they trailed
        # the whole xs stream), while xs3-5 keep ~0.4us arrival margin
        # over their k-chain consumption times
        for kt in range(KH // 2):
            load_xs(kt)
        load_swi(1)
        load_swi(KI + 1)
        for kt in range(KH // 2, KH):
            load_xs(kt)
        for ft in range(2, KI):
            load_swi(ft)
            load_swi(KI + ft)

        named = {}
        named["swo"] = load_rows(nc.sync, swo, KI, "swo")
        named["xe"] = load_rows(nc.sync, xe, KH, "xe")
        named["wi"] = load_rows(nc.sync, wi, KH, "wi", fsplit=2)
        wo8_t = wpool.tile([P, KI, H], FP8, tag="wo8", name="wo8")
        nc.sync.dma_start(wo8_t[:], wo8)

        # warm-up PSUM drain parks in the sync queue's mid-kernel idle window
        # (it must exist so the warm matmuls have a live consumer, but must
        # not sit at the end of any queue where it would extend the tail)
        nc.sync.dma_start(warm_sink[:], warm_out[:])

        # accessors: (ft|ht, kt) -> lhsT AP; x: (kt) -> rhs tile
        sh = dict(
            x=lambda kt: xs_t[kt][0],
            wa=lambda ft, kt: swi_f[ft][:, kt * P:(kt + 1) * P],
            wb=lambda ft, kt: swi_f[KI + ft][:, kt * P:(kt + 1) * P],
            wo=lambda ht, kt: named["swo"][kt][0][:, ht * P:(ht + 1) * P],
        )
        ex = dict(
            x=lambda kt: named["xe"][kt][0],
            wa=lambda ft, kt: named["wi"][kt][0][:, ft * P:(ft + 1) * P],
            wb=lambda ft, kt: named["wi"][kt][1][:, ft * P:(ft + 1) * P],
        )

        # (accessors, y_dram, chunk_off, chunk_sz, silu_on_first)
        chunks = []
        for acc, yd, m, sfirst in ((sh, ys, TS, True), (ex, ye, m_pad, False)):
            off = 0
            for sz in _chunk_sizes(m):
                chunks.append((acc, yd, off, sz, sfirst))
                off += sz

        def emit_wi(c):
            acc, yd, off, sz, sfirst = chunks[c]
            # shared chunk keeps bf16 activations; expert chunks store them
            # as fp8e4 so the Wo pass can run in DoubleRow mode
            adt, atag = (BF16, "act") if sfirst else (FP8, "act8")
            act = apool.tile([P, KI, MAXN], adt, tag=atag, name=atag)[:, :, :sz]
            for ft in range(KI):
                ps_a = psum.tile([P, MAXN], F32, tag="ps", name="ps_a")[:, :sz]
                for kt in range(KH):
                    nc.tensor.matmul(ps_a, lhsT=acc["wa"](ft, kt),
                                     rhs=acc["x"](kt)[:, off:off + sz],
                                     start=(kt == 0), stop=(kt == KH - 1))
                ps_b = psum.tile([P, MAXN], F32, tag="ps", name="ps_b")[:, :sz]
                for kt in range(KH):
                    nc.tensor.matmul(ps_b, lhsT=acc["wb"](ft, kt),
                                     rhs=acc["x"](kt)[:, off:off + sz],
                                     start=(kt == 0), stop=(kt == KH - 1))
                sl = spool.tile([P, MAXN], F32, tag="silu", name="sl")[:, :sz]
                ps_s, ps_m = (ps_a, ps_b) if sfirst else (ps_b, ps_a)
                if USE_SILU:
                    # act = silu(s) * m: one ACT op + one DVE mul; PSUM banks
                    # are freed one op earlier than the sigmoid+2-mul form
                    nc.scalar.activation(sl, ps_s,
                                         mybir.ActivationFunctionType.Silu)
                    nc.vector.tensor_mul(act[:, ft, :], sl, ps_m)
                else:
                    # CoreSim fallback: silu(s) = s * sigmoid(s)
                    tmp = spool.tile([P, MAXN], F32, tag="silu2",
                                     name="tmp")[:, :sz]
                    nc.scalar.activation(sl, ps_s,
                                         mybir.ActivationFunctionType.Sigmoid)
                    nc.vector.tensor_mul(tmp, sl, ps_s)
                    nc.vector.tensor_mul(act[:, ft, :], tmp, ps_m)
            return act

        def emit_wo(c, act, last=False):
            acc, yd, off, sz, sfirst = chunks[c]
            cb = off * KH
            # last chunk: first DMA carries h0-h4 so the tail's critical
            # path is one small copy + one 1-h-tile DMA; its copies run on
            # the (idle) DVE whose PSUM-read copy is ~2x faster than ACT's
            split = (KH - 1 if last else KH // 2) * sz
            yt = ypool.tile([P, KH * MAXN], BF16, tag="y", name="yt")
            # copy (f32 PSUM -> bf16 SBUF) on the Scalar engine (GpSimd has
            # no PSUM access) so DVE-mul throughput isn't what frees PSUM
            # banks; outputs leave as two merged 3-h-tile DMAs so the tail
            # is one small copy + one small DMA
            dma_eng = nc.scalar if last else nc.gpsimd
            for ht in range(KH):
                ps_y = psum.tile([P, MAXN], F32, tag="ps", name="ps_y")[:, :sz]
                hs = slice(ht * P, (ht + 1) * P)
                if sfirst:
                    # shared expert: bf16 9-step k-chain
                    for kt in range(KI):
                        nc.tensor.matmul(ps_y, lhsT=acc["wo"](ht, kt),
                                         rhs=act[:, kt, :],
                                         start=(kt == 0), stop=(kt == KI - 1))
                else:
                    # expert: fp8 DoubleRow over 4 k-tile pairs (256-row
                    # contraction each) + one plain fp8 matmul for k-tile 8
                    for g in range(4):
                        nc.tensor.matmul(
                            ps_y, lhsT=wo8_t[:, 2 * g:2 * g + 2, hs],
                            rhs=act[:, 2 * g:2 * g + 2, :],
                            start=(g == 0), stop=False,
                            perf_mode=mybir.MatmulPerfMode.DoubleRow)
                    nc.tensor.matmul(ps_y, lhsT=wo8_t[:, KI - 1, hs],
                                     rhs=act[:, KI - 1, :],
                                     start=False, stop=True)
                if last:
                    nc.vector.tensor_copy(yt[:, ht * sz:(ht + 1) * sz], ps_y)
                else:
                    nc.scalar.copy(yt[:, ht * sz:(ht + 1) * sz], ps_y)
                if (ht + 1) * sz == split:
                    dma_eng.dma_start(yd[:, cb:cb + split], yt[:, :split])
            dma_eng.dma_start(yd[:, cb + split:cb + KH * sz],
                              yt[:, split:KH * sz])

        # software pipeline: after Wo0 (long-resident shared weights),
        # Wi(c+1) is emitted before Wo(c) so the PE always has independent
        # matmul work while ACT/DVE finish chunk c's SwiGLU.
        n = len(chunks)
        acts = [None] * n
        acts[0] = emit_wi(0)
        if n == 1:
            emit_wo(0, acts[0], last=True)
        else:
            emit_wo(0, acts[0])
            acts[1] = emit_wi(1)
            for c in range(2, n):
                acts[c] = emit_wi(c)
                emit_wo(c - 1, acts[c - 1])
            emit_wo(n - 1, acts[n - 1], last=True)

    nc.compile()
    return nc


def _tile_swi(swiT):
    """(H, 2I) -> (18, P, H): f-tile-major contiguous layout for the device."""
    FI2 = I2 // P
    return np.ascontiguousarray(
        swiT.reshape(KH, P, FI2, P).transpose(2, 1, 0, 3).reshape(FI2, P, H))


def _route(x, gate_w, correction_bias):
    logits = 1.0 / (1.0 + np.exp(-(x @ gate_w.T), dtype=np.float32))  # (T, E)
    sel = logits + correction_bias[None, :]
    order = np.argsort(-sel, axis=1, kind="stable")[:, :TOPK]  # ties -> low index
    w = np.take_along_axis(logits, order, axis=1)
    w = (w / w.sum(axis=1, keepdims=True)).astype(np.float32)
    return order, w


def kernel(**inputs) -> np.ndarray:
    x = np.asarray(inputs["x"], np.float32)
    gate_w = np.asarray(inputs["gate_w"], np.float32)
    bias = np.asarray(inputs["correction_bias"], np.float32)
    Wi = np.asarray(inputs["Wi"], np.float32)
    Wo = np.asarray(inputs["Wo"], np.float32)
    shared_Wi = np.asarray(inputs["shared_Wi"], np.float32)
    shared_Wo = np.asarray(inputs["shared_Wo"], np.float32)

    order, w = _route(x, gate_w, bias)

    idx_per_e, cw_per_e = [], []
    for e in range(E):
        mask = order == e  # (T, K)
        tok = mask.any(axis=1)
        rows = np.nonzero(tok)[0]
        kpos = np.argmax(mask[rows], axis=1)
        idx_per_e.append(rows)
        cw_per_e.append(w[rows, kpos].astype(np.float32))

    mx = max(len(r) for r in idx_per_e)
    m_pad = max(64, mx + (mx & 1))  # exact capacity, kept even for alignment

    bf = ml_dtypes.bfloat16
    f8 = ml_dtypes.float8_e4m3fn
    xT = np.ascontiguousarray(x.T)  # (H, T) f32
    swiT = _tile_swi(shared_Wi.T.astype(bf))             # (18, P, H)
    swoT = np.ascontiguousarray(shared_Wo.T).astype(bf)  # (I, H)

    in_maps = []
    for c in range(N_CORES):
        rows = idx_per_e[c]
        xe = np.zeros((H, m_pad), bf)
        xe[:, :len(rows)] = xT[:, rows].astype(bf)
        # expert Wo: x16 (into fp8e4's well-conditioned range; the host
        # combine divides the weights by 16), partition-major [P, KI, H]
        wo8 = np.ascontiguousarray(
            (Wo[c] * 16.0).reshape(KI, P, H).transpose(1, 0, 2)).astype(f8)
        in_maps.append({
            "xe": xe,
            "wi": Wi[c].astype(bf),                      # (H, 2I)
            "wo8": wo8,                                  # (P, KI, H) fp8
            "xs": np.ascontiguousarray(
                xT[:, c * TS:(c + 1) * TS]).astype(bf),  # (H, TS)
            "swi": swiT,
            "swo": swoT,
        })

    if m_pad not in _BUILD_CACHE:
        _BUILD_CACHE[m_pad] = _build(m_pad)
    nc = _BUILD_CACHE[m_pad]

    _ensure_axon_ntff_hook()
    res = run_bass_kernel_spmd(nc, in_maps, list(range(N_CORES)))
    global LAST_RESULTS
    LAST_RESULTS = res

    def _unflatten(flat, m):
        # [P, KH*m] chunk-major flat -> (m, H): chunk at token-offset off
        # occupies columns [off*KH, (off+sz)*KH); within it, column
        # ht*sz + j holds feature ht*128+p of token off+j
        y = np.empty((m, H), np.float32)
        off = 0
        for sz in _chunk_sizes(m):
            blk = flat[:, off * KH:(off + sz) * KH].reshape(P, KH, sz)
            y[off:off + sz] = blk.transpose(1, 0, 2).reshape(H, sz).T
            off += sz
        return y

    out = np.zeros((T, H), np.float32)
    for c in range(N_CORES):
        r = res.results[c]
        ys_f = _unflatten(np.asarray(r["ys"]).astype(np.float32), TS)
        out[c * TS:(c + 1) * TS] += ys_f
        rows = idx_per_e[c]
        if len(rows):
            ye_f = _unflatten(np.asarray(r["ye"]).astype(np.float32), m_pad)
            out[rows] += ye_f[:len(rows)] * (cw_per_e[c][:, None] / 16.0)
    return out
